# revision 19
# baseline (speedup 1.0000x reference)
"""AdaAttN Trainium2 kernel: B=4, C=256, N=M=4096, f32.

Sharding: 8 cores = batch(4) x N-halves(2). Each core holds full k[b] and
its 2048-column slice of q[b] (plus the other half for instance-norm
stats), computes its slice independently. No collectives.

S is computed TRANSPOSED (m on partitions) so the attention matrix comes
out of the QK matmul already in the layout the AV matmuls need:
  S^T[m,n] = sum_o ke[o,m] qe[o,n]   (lhsT = ke chunk, rhs = qe)
  at = exp(S^T - 64)                 (scalar engine, psum -> sbuf)
  pm[c,n] += se[m,c]^T @ at          (accumulate over all 32 m-chunks)
  p2[c,n] += se2[m,c]^T @ at
  Z[n]    += ones^T @ (preadded at)  (column sums for softmax denom)

Key structural choices vs a naive port:
 - The k-side projection bias (bk - wk_s^T mu_k) contributes a per-n
   offset to the logits (constant over m), which softmax is invariant
   to. It is DROPPED entirely: no kbias matmuls, no ke bias adds; ke
   psum just drains through scalar copies.
 - No Sqrt activations anywhere: rstd = exp(-0.5 ln(v+eps)) and the
   epilogue std = exp(0.5 ln(max(var,floor))). Exp and Ln share one
   activation table set, so the scalar engine never reloads tables
   (reloads cost ~1.3us and used to stall the PE at group boundaries).
 - Column sums use 4 windows of 8 chunks per group: 7 chained pre-adds
   per window (windows alternate DVE/GpSimd ownership) and one
   128x1x512 matmul each, so the PE spends ~1us/group on Z instead of
   ~3.5us.
 - Prologue is arrival-ordered: kf DMAs first and se matmuls chase the
   tiles (se needs no stats); k stats chase on DVE; ql/qo stats follow
   immediately so the q fold is ready right when the (DMA-bound)
   prologue ends. qo is persistent so its DMAs all prefetch.

Engine queues are in-order, so emission order is scheduling; data-paced
work is emitted before dependent-blocked work on each queue.

Biases: b_q folds into the qe bias add; b_s cancels in the variance and
is added to the mean in the epilogue; b_k is dropped (see above).
Softmax uses a fixed shift 64 (logits ~ N(0,16^2)); 1/Z is deferred
past the AV matmuls.
"""

import sys
import types

import numpy as np

B, C, N, M = 4, 256, 4096, 4096
NLOC = N // 2          # per-core n columns
CC = C // 128          # c chunks of 128 partitions
EPS = 1e-5
SHIFT = 64.0           # fixed softmax shift

GN = 512               # n columns per group
NG = NLOC // GN        # groups per core
MC = M // 128          # m chunks (128 wide)
MT = M // 512          # m tiles (512 wide)
CSW = 8                # chunks per column-sum window
NW = MC // CSW         # windows per group
AVD = 3                # AV trails QK by this many chunks
CS_LAG = 5             # chunks between a window's last pre-add and its mm


def _ensure_axon_hooks_stub():
    if "antenv.axon_hooks" in sys.modules:
        return
    try:
        import antenv
    except ImportError:
        return
    mod = types.ModuleType("antenv.axon_hooks")
    mod._HOOK = None
    mod.set_axon_ntff_profile_hook = lambda h: setattr(mod, "_HOOK", h)
    mod.get_axon_ntff_profile_hook = lambda: mod._HOOK
    sys.modules["antenv.axon_hooks"] = mod
    antenv.axon_hooks = mod


def build_bass():
    import concourse.bass as bass
    import concourse.mybir as mybir
    import concourse.tile as tile
    from concourse import bacc
    from concourse.bass import ds, ts
    from contextlib import ExitStack

    f32 = mybir.dt.float32
    f32r = mybir.dt.float32r
    AF = mybir.ActivationFunctionType
    OP = mybir.AluOpType

    class OneTableBacc(bacc.Bacc):
        """Bacc whose act-table placement resolves Exp and Ln to the one
        hardware table set that contains BOTH (natural_log_exp_and_others),
        so the scalar engine never reloads tables mid-kernel. Only the
        choice among valid sets changes; set ids still index the real
        act_info.json list."""

        def insert_act_table_loads(self):
            import bass_rust as _bass_rust
            from concourse.hw_specs import get_activation_tables

            has_activation = any(
                isinstance(i, mybir.InstActivation)
                for b in self.main_func.blocks
                for i in b.instructions
            )
            if not has_activation:
                return
            tables = []
            for name, fns in get_activation_tables(self.m.arch).items():
                fns = set(fns)
                if name != "natural_log_exp_and_others":
                    fns.discard(AF.Exp)
                    fns.discard(AF.Ln)
                tables.append((name, fns))
            _bass_rust.insert_act_table_loads(self, tables)

    nc = OneTableBacc("TRN2", target_bir_lowering=False, debug=False,
                      num_devices=8)

    ql_d = nc.declare_dram_parameter("ql", [C, NLOC], f32, isOutput=False)
    qo_d = nc.declare_dram_parameter("qo", [C, NLOC], f32, isOutput=False)
    kf_d = nc.declare_dram_parameter("kf", [C, M], f32, isOutput=False)
    wqT_d = nc.declare_dram_parameter("wqT", [C, C], f32, isOutput=False)
    wkT_d = nc.declare_dram_parameter("wkT", [C, C], f32, isOutput=False)
    wsT_d = nc.declare_dram_parameter("wsT", [C, C], f32, isOutput=False)
    bq_d = nc.declare_dram_parameter("bq", [C], f32, isOutput=False)
    bs_d = nc.declare_dram_parameter("bs", [C], f32, isOutput=False)
    out_d = nc.declare_dram_parameter("out", [C, NLOC], f32, isOutput=True)

    def r(ap):
        return ap.bitcast(f32r)

    with ExitStack() as ctx:
        tc = ctx.enter_context(tile.TileContext(nc))
        persist = ctx.enter_context(tc.tile_pool(name="persist", bufs=1))
        # 16 slots of [128,512]; kf lives here in the prologue, the slots
        # then recycle as attn tiles in the main loop.
        big = ctx.enter_context(tc.tile_pool(name="big", bufs=16))
        qo_pool = ctx.enter_context(tc.tile_pool(name="qo", bufs=2))
        small = ctx.enter_context(tc.tile_pool(name="small", bufs=4))
        csacc = ctx.enter_context(tc.tile_pool(name="csacc", bufs=2))
        epi = ctx.enter_context(tc.tile_pool(name="epi", bufs=2))
        epi1 = ctx.enter_context(tc.tile_pool(name="epi1", bufs=1))
        invp = ctx.enter_context(tc.tile_pool(name="invp", bufs=1))
        psum_qk = ctx.enter_context(tc.tile_pool(name="psum_qk", bufs=3,
                                                 space="PSUM"))
        psum_av = ctx.enter_context(tc.tile_pool(name="psum_av", bufs=4,
                                                 space="PSUM"))
        psum_cs = ctx.enter_context(tc.tile_pool(name="psum_cs", bufs=1,
                                                 space="PSUM"))

        # ---- persistent tensors ----
        ql_sb = persist.tile([128, CC, NLOC], f32r)
        qe_sb = persist.tile([128, CC, NLOC], f32r)
        ke_sb = persist.tile([128, CC, M], f32r)
        se_sb = persist.tile([128, MC, C], f32r)
        se2_sb = persist.tile([128, MC, C], f32r)
        wqT_sb = persist.tile([128, CC, C], f32r)   # becomes rs_q-scaled
        wkT_sb = persist.tile([128, CC, C], f32r)   # becomes rs_k-scaled
        wsT_sb = persist.tile([128, CC, C], f32r)
        bq_sb = persist.tile([128, CC], f32)
        bs_sb = persist.tile([128, CC], f32)
        qbias_sb = persist.tile([128, CC], f32)
        ones_col = persist.tile([128, 1], f32r)
        eps_t = persist.tile([128, 1], f32)
        shift_t = persist.tile([128, 1], f32)

        nc.vector.memset(eps_t, EPS)
        nc.vector.memset(shift_t, -SHIFT)
        nc.gpsimd.memset(ones_col.bitcast(f32), 1.0)

        # ---- input DMAs: weights, then kf (se/stats chase it), then q ----
        for cc in range(CC):
            nc.sync.dma_start(wsT_sb[:, cc, :], r(wsT_d[ts(cc, 128), :]))
            nc.sync.dma_start(wkT_sb[:, cc, :], r(wkT_d[ts(cc, 128), :]))
            nc.sync.dma_start(wqT_sb[:, cc, :], r(wqT_d[ts(cc, 128), :]))
        nc.sync.dma_start(bq_sb, bq_d.rearrange("(o p) -> p o", p=128))
        nc.sync.dma_start(bs_sb, bs_d.rearrange("(o p) -> p o", p=128))
        kf_t = {}
        for mt in range(MT):
            for cc in range(CC):
                t = big.tile([128, 512], f32r, tag="big", name=f"kf{cc}_{mt}")
                nc.sync.dma_start(t, r(kf_d[ts(cc, 128), ts(mt, 512)]))
                kf_t[cc, mt] = t
        for j in range(4):
            for cc in range(CC):
                nc.sync.dma_start(ql_sb[:, cc, ts(j, 512)],
                                  r(ql_d[ts(cc, 128), ts(j, 512)]))
        qo_t = {}
        for j in range(4):
            for cc in range(CC):
                t = qo_pool.tile([128, 512], f32, tag="qo")
                nc.sync.dma_start(t, qo_d[ts(cc, 128), ts(j, 512)])
                qo_t[cc, j] = t

        # ---- PE: se = kf^T @ ws (m, c) chases the kf DMAs; no stats dep.
        # Two m-chunks pack into one psum tile so each scalar copy (and
        # each gpsimd square) moves 512 columns — the scalar queue would
        # otherwise pace the whole prologue.
        for mp in range(MC // 2):
            ps = psum_qk.tile([128, 512], f32, tag="qk")
            for half in range(2):
                mc = 2 * mp + half
                for cc in range(CC):
                    nc.tensor.matmul(ps[:, ts(half, C)],
                                     kf_t[cc, mc // 4][:, ts(mc % 4, 128)],
                                     wsT_sb[:, cc, :],
                                     start=(cc == 0), stop=(cc == CC - 1))
            nc.scalar.copy(se_sb[:, ts(mp, 2), :], ps)
            nc.gpsimd.tensor_tensor(se2_sb[:, ts(mp, 2), :],
                                    se_sb[:, ts(mp, 2), :].bitcast(f32),
                                    se_sb[:, ts(mp, 2), :].bitcast(f32),
                                    OP.mult)

        # ---- DVE: k stats chase the kf DMAs ----
        kstats_t = []
        for cc in range(CC):
            stats = small.tile([128, 8, 6], f32, tag="kstats")
            kstats_t.append(stats)
        for mt in range(MT):
            for cc in range(CC):
                nc.vector.bn_stats(out=kstats_t[cc][:, mt, :],
                                   in_=kf_t[cc, mt].bitcast(f32))
        rs_k = []
        for cc in range(CC):
            mv = small.tile([128, 2], f32, tag="kmv")
            nc.vector.bn_aggr(out=mv, in_=kstats_t[cc])
            # rstd = exp(-0.5 ln(v + eps)); keeps the scalar engine on the
            # exp/ln table set (no reload before the softmax exps)
            lnv = small.tile([128, 1], f32, tag="klnv")
            nc.scalar.activation(out=lnv, in_=mv[:, 1:2], func=AF.Ln,
                                 bias=eps_t, scale=1.0)
            rstd = small.tile([128, 1], f32, tag="krstd")
            nc.scalar.activation(out=rstd, in_=lnv, func=AF.Exp, scale=-0.5)
            rs_k.append(rstd)
        for cc in range(CC):
            nc.vector.tensor_scalar_mul(wkT_sb[:, cc, :],
                                        wkT_sb[:, cc, :].bitcast(f32), rs_k[cc])

        # ---- PE: ke = wk_s^T @ kf (o, m); drains via scalar copies.
        # The k-side projection bias is per-n in the logits -> dropped.
        for oc in range(CC):
            for mt in range(MT):
                ps = psum_qk.tile([128, 512], f32, tag="qk")
                for cc in range(CC):
                    nc.tensor.matmul(ps, wkT_sb[:, cc, ts(oc, 128)],
                                     kf_t[cc, mt],
                                     start=(cc == 0), stop=(cc == CC - 1))
                nc.scalar.copy(ke_sb[:, oc, ts(mt, 512)], ps)

        # ---- DVE: q stats (both halves) chase the q DMAs ----
        qstats_t = []
        for cc in range(CC):
            stats = small.tile([128, 8, 6], f32, tag="qstats")
            qstats_t.append(stats)
        for j in range(4):
            for cc in range(CC):
                nc.vector.bn_stats(out=qstats_t[cc][:, j, :],
                                   in_=ql_sb[:, cc, ts(j, 512)].bitcast(f32))
        for j in range(4):
            for cc in range(CC):
                nc.vector.bn_stats(out=qstats_t[cc][:, 4 + j, :],
                                   in_=qo_t[cc, j])
        mu_q, rs_q, nmu_q = [], [], []
        for cc in range(CC):
            mv = small.tile([128, 2], f32, tag="qmv")
            nc.vector.bn_aggr(out=mv, in_=qstats_t[cc])
            negmu = small.tile([128, 2], f32r, tag="qnegmu")
            nc.vector.tensor_scalar_mul(negmu, mv[:, 0:2], -1.0)
            mu = small.tile([128, 1], f32, tag="qmu")
            nc.vector.tensor_copy(out=mu, in_=mv[:, 0:1])
            lnv = small.tile([128, 1], f32, tag="qlnv")
            nc.scalar.activation(out=lnv, in_=mv[:, 1:2], func=AF.Ln,
                                 bias=eps_t, scale=1.0)
            rstd = small.tile([128, 1], f32, tag="qrstd")
            nc.scalar.activation(out=rstd, in_=lnv, func=AF.Exp, scale=-0.5)
            mu_q.append(mu)
            nmu_q.append(negmu)
            rs_q.append(rstd)
        for cc in range(CC):
            nc.vector.tensor_scalar_mul(wqT_sb[:, cc, :],
                                        wqT_sb[:, cc, :].bitcast(f32), rs_q[cc])

        # ---- qbias = bq + wq_s^T(-mu_q) ----
        qbias_ps = []
        for oc in range(CC):
            pb = psum_qk.tile([128, 512], f32, tag="qk", name=f"qb{oc}")
            for cc in range(CC):
                nc.tensor.matmul(pb[:, 0:2], wqT_sb[:, cc, ts(oc, 128)],
                                 nmu_q[cc], start=(cc == 0), stop=(cc == CC - 1))
            nc.vector.tensor_tensor(qbias_sb[:, oc:oc + 1], pb[:, 0:1],
                                    bq_sb[:, oc:oc + 1], OP.add)

        # ---- qe (o, n): nt=0 first so group 0 can start; rest in-loop ----
        def qe_tile(nt):
            for oc in range(CC):
                ps = psum_qk.tile([128, 512], f32, tag="qk")
                for cc in range(CC):
                    nc.tensor.matmul(ps, wqT_sb[:, cc, ts(oc, 128)],
                                     ql_sb[:, cc, ts(nt, 512)],
                                     start=(cc == 0), stop=(cc == CC - 1))
                nc.vector.tensor_scalar_add(qe_sb[:, oc, ts(nt, 512)], ps,
                                            qbias_sb[:, oc:oc + 1])
        qe_tile(0)

        # ---- main loop ----
        pend = {}

        def epilogue_tail(g):
            (var2, std, mean, invb) = pend.pop(g)
            # std = exp(0.5 ln var); Ln runs in place on var2 (last use)
            nc.scalar.activation(out=var2, in_=var2, func=AF.Ln)
            nc.scalar.activation(out=std, in_=var2, func=AF.Exp, scale=0.5)
            for cc in range(CC):
                qnt = epi1.tile([128, 512], f32, tag="qnt")
                nc.vector.tensor_scalar(out=qnt,
                                        in0=ql_sb[:, cc, ts(g, GN)].bitcast(f32),
                                        scalar1=mu_q[cc], scalar2=rs_q[cc],
                                        op0=OP.subtract, op1=OP.mult)
                t1 = epi.tile([128, 512], f32, tag="t1")
                nc.vector.tensor_tensor(t1, qnt, std[:, cc, :], OP.mult)
                # out = (qn*std + bs) + mean
                nc.vector.scalar_tensor_tensor(
                    out=t1, in0=t1, scalar=bs_sb[:, cc:cc + 1], in1=mean[cc],
                    op0=OP.add, op1=OP.add)
                nc.sync.dma_start(out_d[ts(cc, 128), ts(g, GN)], t1)

        for g in range(NG):
            pm = [psum_av.tile([128, GN], f32, tag="av", name=f"pm{g}_{i}")
                  for i in range(CC)]
            p2 = [psum_av.tile([128, GN], f32, tag="av", name=f"p2{g}_{i}")
                  for i in range(CC)]
            pcs = psum_cs.tile([1, GN], f32, tag="cs", name=f"pcs{g}")
            at_t = {}
            accs = {}

            def preadd(j, at):
                # column-sum pre-adds: window w owns chunks 8w..8w+7,
                # split into two 4-chunk branches that run on DVE and
                # GpSimd concurrently (3 chained adds each), then one
                # combine. Keeps the chain latency after the window's
                # last chunk to ~2 adds so cs_mm never stalls the PE.
                w, ph = divmod(j, CSW)
                half, hp = divmod(ph, 4)
                engA = nc.gpsimd if w % 2 == 0 else nc.vector
                engB = nc.vector if w % 2 == 0 else nc.gpsimd
                eng = engA if half == 0 else engB
                if hp == 0:
                    at_first[w, half] = at
                elif hp == 1:
                    acc = csacc.tile([128, GN], f32r,
                                     tag="accA" if half == 0 else "accB",
                                     bufs=2 if half == 0 else 1,
                                     name=f"acc{g}_{w}_{half}")
                    halfacc[w, half] = acc
                    eng.tensor_tensor(acc, at_first.pop((w, half)).bitcast(f32),
                                      at.bitcast(f32), OP.add)
                else:
                    eng.tensor_tensor(halfacc[w, half],
                                      halfacc[w, half].bitcast(f32),
                                      at.bitcast(f32), OP.add)
                if ph == CSW - 1:
                    a = halfacc.pop((w, 0))
                    b = halfacc.pop((w, 1))
                    engA.tensor_tensor(a, a.bitcast(f32), b.bitcast(f32),
                                       OP.add)
                    accs[w] = a

            def av_chunk(j):
                at = at_t.pop(j)
                first, last = (j == 0), (j == MC - 1)
                nc.tensor.matmul(pm[0], se_sb[:, j, 0:128], at,
                                 start=first, stop=last)
                nc.tensor.matmul(pm[1], se_sb[:, j, 128:256], at,
                                 start=first, stop=last)
                nc.tensor.matmul(p2[0], se2_sb[:, j, 0:128], at,
                                 start=first, stop=last)
                nc.tensor.matmul(p2[1], se2_sb[:, j, 128:256], at,
                                 start=first, stop=last)
                preadd(j, at)

            def cs_mm(w):
                nc.tensor.matmul(pcs, ones_col, accs.pop(w),
                                 start=(w == 0), stop=(w == NW - 1))

            at_first = {}
            halfacc = {}
            for mc in range(MC + AVD):
                if g == 0 and 4 <= mc < 7:
                    qe_tile(mc - 3)
                if mc == 10 and (g - 1) in pend:
                    epilogue_tail(g - 1)
                if mc < MC:
                    ps = psum_qk.tile([128, GN], f32, tag="qk",
                                      name=f"qk{g}_{mc}")
                    for cc in range(CC):
                        nc.tensor.matmul(ps, ke_sb[:, cc, ts(mc, 128)],
                                         qe_sb[:, cc, ts(g, GN)],
                                         start=(cc == 0), stop=(cc == CC - 1))
                    at = big.tile([128, GN], f32r, tag="big",
                                  name=f"at{g}_{mc}")
                    nc.scalar.activation(out=at, in_=ps, func=AF.Exp,
                                         bias=shift_t)
                    at_t[mc] = at
                if mc >= AVD:
                    av_chunk(mc - AVD)
                    jj = mc - AVD - CS_LAG
                    if jj >= 0 and jj % CSW == CSW - 1 and jj // CSW in accs:
                        cs_mm(jj // CSW)
            for w in sorted(accs):
                cs_mm(w)

            # ---- epilogue part 1 ----
            invrow = invp.tile([1, GN], f32, tag="invrow", name=f"ivr{g}")
            nc.vector.reciprocal_approx_fast(out=invrow, in_=pcs[0:1, :])
            invb = invp.tile([128, GN], f32, tag="invb", name=f"ivb{g}")
            nc.gpsimd.partition_broadcast(invb, invrow)
            mean = [epi.tile([128, GN], f32, tag="mean", name=f"u{g}_{cc}")
                    for cc in range(CC)]
            m2n = [epi1.tile([128, GN], f32, tag="m2n", name=f"v{g}_{cc}")
                   for cc in range(CC)]
            var2 = epi1.tile([128, CC, GN], f32, tag="var2", name=f"var{g}")
            std = epi1.tile([128, CC, GN], f32, tag="std", name=f"std{g}")
            if g < NG - 1:
                # drain psum with plain copies (pm first: AV(g+1) needs
                # those banks soonest), then normalize in place
                for cc in range(CC):
                    nc.vector.tensor_copy(out=mean[cc], in_=pm[cc])
                for cc in range(CC):
                    nc.vector.tensor_copy(out=m2n[cc], in_=p2[cc])
                for cc in range(CC):
                    nc.vector.tensor_tensor(mean[cc], mean[cc], invb, OP.mult)
                    nc.vector.tensor_tensor(m2n[cc], m2n[cc], invb, OP.mult)
                for cc in range(CC):
                    msq = epi1.tile([128, GN], f32, tag="msq")
                    nc.vector.tensor_tensor(msq, mean[cc], mean[cc], OP.mult)
                    nc.vector.tensor_tensor(var2[:, cc, :], m2n[cc], msq,
                                            OP.subtract)
                # floor > 0 so ln/exp of an exact-zero variance stays finite
                nc.vector.tensor_scalar_max(var2, var2, 1e-30)
                pend[g] = (var2, std, mean, invb)
            else:
                # last group: latency-optimized per-cc pipeline, psum read
                # directly, ln/exp std and qnt interleaved, DMA per cc ASAP
                qnt = []
                for cc in range(CC):
                    qn = epi1.tile([128, GN], f32, tag="qnt" if cc == 0 else "qnt1",
                                   name=f"qnt{g}_{cc}")
                    nc.vector.tensor_scalar(
                        out=qn, in0=ql_sb[:, cc, ts(g, GN)].bitcast(f32),
                        scalar1=mu_q[cc], scalar2=rs_q[cc],
                        op0=OP.subtract, op1=OP.mult)
                    qnt.append(qn)
                for cc in range(CC):
                    nc.vector.tensor_tensor(mean[cc], pm[cc], invb, OP.mult)
                    nc.vector.tensor_tensor(m2n[cc], p2[cc], invb, OP.mult)
                    msq = epi1.tile([128, GN], f32, tag="msq")
                    # mean^2 on GpSimd, overlapping the DVE chain
                    nc.gpsimd.tensor_tensor(msq, mean[cc], mean[cc], OP.mult)
                    nc.vector.tensor_tensor(var2[:, cc, :], m2n[cc], msq,
                                            OP.subtract)
                    nc.vector.tensor_scalar_max(var2[:, cc, :],
                                                var2[:, cc, :], 1e-30)
                    nc.scalar.activation(out=var2[:, cc, :],
                                         in_=var2[:, cc, :], func=AF.Ln)
                    nc.scalar.activation(out=std[:, cc, :], in_=var2[:, cc, :],
                                         func=AF.Exp, scale=0.5)
                for cc in range(CC):
                    t1 = epi.tile([128, 512], f32, tag="t1")
                    nc.vector.tensor_tensor(t1, qnt[cc], std[:, cc, :],
                                            OP.mult)
                    nc.vector.scalar_tensor_tensor(
                        out=t1, in0=t1, scalar=bs_sb[:, cc:cc + 1],
                        in1=mean[cc], op0=OP.add, op1=OP.add)
                    # exposed tail: split across 4 DMA queues
                    for s in range(4):
                        nc.sync.dma_start(
                            out_d[ts(cc, 128), ds(g * GN + s * 128, 128)],
                            t1[:, ts(s, 128)])

    nc.finalize()
    return nc


_NC = None


def _get_nc():
    global _NC
    if _NC is None:
        _ensure_axon_hooks_stub()
        _NC = build_bass()
    return _NC


def make_in_maps(q, k, w_q, b_q, w_k, b_k, w_s, b_s):
    q = np.ascontiguousarray(np.asarray(q, dtype=np.float32))
    k = np.ascontiguousarray(np.asarray(k, dtype=np.float32))
    wqT = np.ascontiguousarray(np.asarray(w_q, np.float32).T)
    wkT = np.ascontiguousarray(np.asarray(w_k, np.float32).T)
    wsT = np.ascontiguousarray(np.asarray(w_s, np.float32).T)
    bq = np.ascontiguousarray(np.asarray(b_q, np.float32))
    bs = np.ascontiguousarray(np.asarray(b_s, np.float32))
    in_maps = []
    for core in range(8):
        b, h = divmod(core, 2)
        in_maps.append({
            "ql": np.ascontiguousarray(q[b][:, h * NLOC:(h + 1) * NLOC]),
            "qo": np.ascontiguousarray(q[b][:, (1 - h) * NLOC:(2 - h) * NLOC]),
            "kf": np.ascontiguousarray(k[b]),
            "wqT": wqT, "wkT": wkT, "wsT": wsT,
            "bq": bq, "bs": bs,
        })
    return in_maps


def kernel(**inputs):
    _ensure_axon_hooks_stub()
    from concourse.bass_utils import run_bass_kernel_spmd

    nc = _get_nc()
    in_maps = make_in_maps(**inputs)
    res = run_bass_kernel_spmd(nc, in_maps, core_ids=list(range(8)))
    out = np.empty((B, C, N), np.float32)
    for core in range(8):
        b, h = divmod(core, 2)
        out[b][:, h * NLOC:(h + 1) * NLOC] = res.results[core]["out"]
    return out


if __name__ == "__main__":
    import reference
    inputs = {k_: np.asarray(v) for k_, v in reference.setup_inputs().items()}
    expected = np.asarray(reference.reference(**inputs))
    actual = kernel(**inputs)
    err = np.linalg.norm(actual - expected) / np.linalg.norm(expected)
    print("Relative error:", err)


# revision 31
# speedup vs baseline: 1.1037x; 1.1037x over previous
"""AdaAttN Trainium2 kernel: B=4, C=256, N=M=4096, f32.

Sharding: 8 cores = batch(4) x N-halves(2). Each core holds full k[b] and
its 2048-column slice of q[b] (plus the other half for instance-norm
stats), computes its slice independently. No collectives.

S is computed TRANSPOSED (m on partitions) so the attention matrix comes
out of the QK matmul already in the layout the AV matmuls need:
  S^T[m,n] = sum_o ke[o,m] qe[o,n]   (lhsT = ke chunk, rhs = qe)
  at = exp(S^T - 64)                 (scalar engine, psum -> sbuf)
  pm[c,n] += se[m,c]^T @ at          (accumulate over all 32 m-chunks)
  p2[c,n] += se2[m,c]^T @ at
  Z[n]    += ones^T @ (preadded at)  (column sums for softmax denom)

Key structural choices vs a naive port:
 - The k-side projection bias (bk - wk_s^T mu_k) contributes a per-n
   offset to the logits (constant over m), which softmax is invariant
   to. It is DROPPED entirely: no kbias matmuls, no ke bias adds; ke
   psum just drains through scalar copies.
 - No Sqrt activations anywhere: rstd = exp(-0.5 ln(v+eps)) and the
   epilogue std = exp(0.5 ln(max(var,floor))). Exp and Ln share one
   activation table set, so the scalar engine never reloads tables
   (reloads cost ~1.3us and used to stall the PE at group boundaries).
 - Column sums use 4 windows of 8 chunks per group: 7 chained pre-adds
   per window (windows alternate DVE/GpSimd ownership) and one
   128x1x512 matmul each, so the PE spends ~1us/group on Z instead of
   ~3.5us.
 - Prologue is arrival-ordered: kf DMAs first and se matmuls chase the
   tiles (se needs no stats); k stats chase on DVE; ql/qo stats follow
   immediately so the q fold is ready right when the (DMA-bound)
   prologue ends. qo is persistent so its DMAs all prefetch.

Engine queues are in-order, so emission order is scheduling; data-paced
work is emitted before dependent-blocked work on each queue.

Biases: b_q folds into the qe bias add; b_s cancels in the variance and
is added to the mean in the epilogue; b_k is dropped (see above).
Softmax uses a fixed shift 64 (logits ~ N(0,16^2)); 1/Z is deferred
past the AV matmuls.
"""

import sys
import types

import numpy as np

B, C, N, M = 4, 256, 4096, 4096
NLOC = N // 2          # per-core n columns
CC = C // 128          # c chunks of 128 partitions
EPS = 1e-5
SHIFT = 64.0           # fixed softmax shift

GN = 512               # n columns per group
NG = NLOC // GN        # groups per core
MC = M // 128          # m chunks (128 wide)
MT = M // 512          # m tiles (512 wide)
CSW = 8                # chunks per column-sum window
NW = MC // CSW         # windows per group
AVD = 8                # AV trails QK by this many chunks; the runway also
                       # covers the prior group's psum-bank release
CS_LAG = 5             # chunks between a window's last pre-add and its mm


def _ensure_axon_hooks_stub():
    if "antenv.axon_hooks" in sys.modules:
        return
    try:
        import antenv
    except ImportError:
        return
    mod = types.ModuleType("antenv.axon_hooks")
    mod._HOOK = None
    mod.set_axon_ntff_profile_hook = lambda h: setattr(mod, "_HOOK", h)
    mod.get_axon_ntff_profile_hook = lambda: mod._HOOK
    sys.modules["antenv.axon_hooks"] = mod
    antenv.axon_hooks = mod


def build_bass():
    import concourse.bass as bass
    import concourse.mybir as mybir
    import concourse.tile as tile
    from concourse import bacc
    from concourse.bass import ds, ts
    from contextlib import ExitStack

    f32 = mybir.dt.float32
    f32r = mybir.dt.float32r
    AF = mybir.ActivationFunctionType
    OP = mybir.AluOpType

    class OneTableBacc(bacc.Bacc):
        """Bacc whose act-table placement resolves Exp and Ln to the one
        hardware table set that contains BOTH (natural_log_exp_and_others),
        so the scalar engine never reloads tables mid-kernel. Only the
        choice among valid sets changes; set ids still index the real
        act_info.json list."""

        def insert_act_table_loads(self):
            import bass_rust as _bass_rust
            from concourse.hw_specs import get_activation_tables

            has_activation = any(
                isinstance(i, mybir.InstActivation)
                for b in self.main_func.blocks
                for i in b.instructions
            )
            if not has_activation:
                return
            tables = []
            for name, fns in get_activation_tables(self.m.arch).items():
                fns = set(fns)
                if name != "natural_log_exp_and_others":
                    fns.discard(AF.Exp)
                    fns.discard(AF.Ln)
                tables.append((name, fns))
            _bass_rust.insert_act_table_loads(self, tables)

    nc = OneTableBacc("TRN2", target_bir_lowering=False, debug=False,
                      num_devices=8)

    ql_d = nc.declare_dram_parameter("ql", [C, NLOC], f32, isOutput=False)
    qo_d = nc.declare_dram_parameter("qo", [C, NLOC], f32, isOutput=False)
    kf_d = nc.declare_dram_parameter("kf", [C, M], f32, isOutput=False)
    wqT_d = nc.declare_dram_parameter("wqT", [C, C], f32, isOutput=False)
    wkT_d = nc.declare_dram_parameter("wkT", [C, C], f32, isOutput=False)
    wsT_d = nc.declare_dram_parameter("wsT", [C, C], f32, isOutput=False)
    bq_d = nc.declare_dram_parameter("bq", [C], f32, isOutput=False)
    bs_d = nc.declare_dram_parameter("bs", [C], f32, isOutput=False)
    out_d = nc.declare_dram_parameter("out", [C, NLOC], f32, isOutput=True)

    def r(ap):
        return ap.bitcast(f32r)

    with ExitStack() as ctx:
        tc = ctx.enter_context(tile.TileContext(nc))
        persist = ctx.enter_context(tc.tile_pool(name="persist", bufs=1))
        # 16 slots of [128,512]; kf lives here in the prologue, the slots
        # then recycle as attn tiles in the main loop.
        big = ctx.enter_context(tc.tile_pool(name="big", bufs=16))
        qo_pool = ctx.enter_context(tc.tile_pool(name="qo", bufs=3))
        small = ctx.enter_context(tc.tile_pool(name="small", bufs=4))
        csacc = ctx.enter_context(tc.tile_pool(name="csacc", bufs=2))
        epi = ctx.enter_context(tc.tile_pool(name="epi", bufs=2))
        epi1 = ctx.enter_context(tc.tile_pool(name="epi1", bufs=1))
        invp = ctx.enter_context(tc.tile_pool(name="invp", bufs=1))
        psum_qk = ctx.enter_context(tc.tile_pool(name="psum_qk", bufs=3,
                                                 space="PSUM"))
        psum_av = ctx.enter_context(tc.tile_pool(name="psum_av", bufs=4,
                                                 space="PSUM"))
        psum_cs = ctx.enter_context(tc.tile_pool(name="psum_cs", bufs=1,
                                                 space="PSUM"))

        # ---- persistent tensors ----
        ql_sb = persist.tile([128, CC, NLOC], f32r)
        qe_sb = persist.tile([128, CC, NLOC], f32r)
        ke_sb = persist.tile([128, CC, M], f32r)
        se_sb = persist.tile([128, MC, C], f32r)
        se2_sb = persist.tile([128, MC, C], f32r)
        wqT_sb = persist.tile([128, CC, C], f32r)   # becomes rs_q-scaled
        wkT_sb = persist.tile([128, CC, C], f32r)   # becomes rs_k-scaled
        wsT_sb = persist.tile([128, CC, C], f32r)
        bq_sb = persist.tile([128, CC], f32)
        bs_sb = persist.tile([128, CC], f32)
        qbias_sb = persist.tile([128, CC], f32)
        ones_col = persist.tile([128, 1], f32r)
        ones_row = persist.tile([1, 128], f32)
        eps_t = persist.tile([128, 1], f32)
        shift_t = persist.tile([128, 1], f32)

        nc.vector.memset(eps_t, EPS)
        nc.vector.memset(shift_t, -SHIFT)
        nc.gpsimd.memset(ones_col.bitcast(f32), 1.0)
        nc.gpsimd.memset(ones_row, 1.0)

        # ---- input DMAs: weights, then kf (se/stats chase it), then q ----
        for cc in range(CC):
            nc.sync.dma_start(wsT_sb[:, cc, :], r(wsT_d[ts(cc, 128), :]))
            nc.sync.dma_start(wkT_sb[:, cc, :], r(wkT_d[ts(cc, 128), :]))
            nc.sync.dma_start(wqT_sb[:, cc, :], r(wqT_d[ts(cc, 128), :]))
        nc.sync.dma_start(bq_sb, bq_d.rearrange("(o p) -> p o", p=128))
        nc.sync.dma_start(bs_sb, bs_d.rearrange("(o p) -> p o", p=128))
        kf_t = {}
        for mt in range(MT):
            for cc in range(CC):
                t = big.tile([128, 512], f32r, tag="big", name=f"kf{cc}_{mt}")
                nc.sync.dma_start(t, r(kf_d[ts(cc, 128), ts(mt, 512)]))
                kf_t[cc, mt] = t
        for j in range(4):
            for cc in range(CC):
                nc.sync.dma_start(ql_sb[:, cc, ts(j, 512)],
                                  r(ql_d[ts(cc, 128), ts(j, 512)]))
        qo_t = {}
        for j in range(4):
            for cc in range(CC):
                t = qo_pool.tile([128, 512], f32, tag="qo")
                nc.sync.dma_start(t, qo_d[ts(cc, 128), ts(j, 512)])
                qo_t[cc, j] = t

        # ---- PE: se = kf^T @ ws (m, c) chases the kf DMAs; no stats dep.
        # Two m-chunks pack into one psum tile so each scalar copy (and
        # each gpsimd square) moves 512 columns — the scalar queue would
        # otherwise pace the whole prologue.
        for mp in range(MC // 2):
            ps = psum_qk.tile([128, 512], f32, tag="qk")
            for half in range(2):
                mc = 2 * mp + half
                for cc in range(CC):
                    nc.tensor.matmul(ps[:, ts(half, C)],
                                     kf_t[cc, mc // 4][:, ts(mc % 4, 128)],
                                     wsT_sb[:, cc, :],
                                     start=(cc == 0), stop=(cc == CC - 1))
            nc.scalar.copy(se_sb[:, ts(mp, 2), :], ps)
            nc.gpsimd.tensor_tensor(se2_sb[:, ts(mp, 2), :],
                                    se_sb[:, ts(mp, 2), :].bitcast(f32),
                                    se_sb[:, ts(mp, 2), :].bitcast(f32),
                                    OP.mult)

        # ---- DVE: k stats chase the kf DMAs ----
        kstats_t = []
        for cc in range(CC):
            stats = small.tile([128, 8, 6], f32, tag="kstats")
            kstats_t.append(stats)
        for mt in range(MT):
            for cc in range(CC):
                nc.vector.bn_stats(out=kstats_t[cc][:, mt, :],
                                   in_=kf_t[cc, mt].bitcast(f32))
        rs_k = []
        for cc in range(CC):
            mv = small.tile([128, 2], f32, tag="kmv")
            nc.vector.bn_aggr(out=mv, in_=kstats_t[cc])
            # rstd = exp(-0.5 ln(v + eps)); keeps the scalar engine on the
            # exp/ln table set (no reload before the softmax exps)
            lnv = small.tile([128, 1], f32, tag="klnv")
            nc.scalar.activation(out=lnv, in_=mv[:, 1:2], func=AF.Ln,
                                 bias=eps_t, scale=1.0)
            rstd = small.tile([128, 1], f32, tag="krstd")
            nc.scalar.activation(out=rstd, in_=lnv, func=AF.Exp, scale=-0.5)
            rs_k.append(rstd)
        for cc in range(CC):
            nc.vector.tensor_scalar_mul(wkT_sb[:, cc, :],
                                        wkT_sb[:, cc, :].bitcast(f32), rs_k[cc])

        # ---- PE: ke = wk_s^T @ kf (o, m); drains via scalar copies.
        # The k-side projection bias is per-n in the logits -> dropped.
        for oc in range(CC):
            for mt in range(MT):
                ps = psum_qk.tile([128, 512], f32, tag="qk")
                for cc in range(CC):
                    nc.tensor.matmul(ps, wkT_sb[:, cc, ts(oc, 128)],
                                     kf_t[cc, mt],
                                     start=(cc == 0), stop=(cc == CC - 1))
                nc.scalar.copy(ke_sb[:, oc, ts(mt, 512)], ps)

        # ---- DVE: q stats (both halves) chase the q DMAs ----
        qstats_t = []
        for cc in range(CC):
            stats = small.tile([128, 8, 6], f32, tag="qstats")
            qstats_t.append(stats)
        for j in range(4):
            for cc in range(CC):
                nc.vector.bn_stats(out=qstats_t[cc][:, j, :],
                                   in_=ql_sb[:, cc, ts(j, 512)].bitcast(f32))
        for j in range(4):
            for cc in range(CC):
                nc.vector.bn_stats(out=qstats_t[cc][:, 4 + j, :],
                                   in_=qo_t[cc, j])
        mu_q, rs_q, nmu_q = [], [], []
        for cc in range(CC):
            mv = small.tile([128, 2], f32, tag="qmv")
            nc.vector.bn_aggr(out=mv, in_=qstats_t[cc])
            negmu = small.tile([128, 2], f32r, tag="qnegmu")
            nc.vector.tensor_scalar_mul(negmu, mv[:, 0:2], -1.0)
            mu = small.tile([128, 1], f32, tag="qmu")
            nc.vector.tensor_copy(out=mu, in_=mv[:, 0:1])
            lnv = small.tile([128, 1], f32, tag="qlnv")
            nc.scalar.activation(out=lnv, in_=mv[:, 1:2], func=AF.Ln,
                                 bias=eps_t, scale=1.0)
            rstd = small.tile([128, 1], f32, tag="qrstd")
            nc.scalar.activation(out=rstd, in_=lnv, func=AF.Exp, scale=-0.5)
            mu_q.append(mu)
            nmu_q.append(negmu)
            rs_q.append(rstd)
        for cc in range(CC):
            nc.vector.tensor_scalar_mul(wqT_sb[:, cc, :],
                                        wqT_sb[:, cc, :].bitcast(f32), rs_q[cc])

        # ---- qbias = bq + wq_s^T(-mu_q) ----
        qbias_ps = []
        for oc in range(CC):
            pb = psum_qk.tile([128, 512], f32, tag="qk", name=f"qb{oc}")
            for cc in range(CC):
                nc.tensor.matmul(pb[:, 0:2], wqT_sb[:, cc, ts(oc, 128)],
                                 nmu_q[cc], start=(cc == 0), stop=(cc == CC - 1))
            nc.vector.tensor_tensor(qbias_sb[:, oc:oc + 1], pb[:, 0:1],
                                    bq_sb[:, oc:oc + 1], OP.add)

        # ---- qe (o, n): nt=0 first so group 0 can start; rest in-loop ----
        def qe_tile(nt):
            for oc in range(CC):
                ps = psum_qk.tile([128, 512], f32, tag="qk")
                for cc in range(CC):
                    nc.tensor.matmul(ps, wqT_sb[:, cc, ts(oc, 128)],
                                     ql_sb[:, cc, ts(nt, 512)],
                                     start=(cc == 0), stop=(cc == CC - 1))
                nc.vector.tensor_scalar_add(qe_sb[:, oc, ts(nt, 512)], ps,
                                            qbias_sb[:, oc:oc + 1])
        qe_tile(0)

        # ---- main loop ----
        pend = {}

        def epilogue_tail(g):
            (var2, std, mean, invb) = pend.pop(g)
            # std = exp(0.5 ln var); Ln runs in place on var2 (last use)
            nc.scalar.activation(out=var2, in_=var2, func=AF.Ln)
            nc.scalar.activation(out=std, in_=var2, func=AF.Exp, scale=0.5)
            for cc in range(CC):
                qnt = epi1.tile([128, 512], f32, tag="qnt")
                nc.vector.tensor_scalar(out=qnt,
                                        in0=ql_sb[:, cc, ts(g, GN)].bitcast(f32),
                                        scalar1=mu_q[cc], scalar2=rs_q[cc],
                                        op0=OP.subtract, op1=OP.mult)
                t1 = epi.tile([128, 512], f32, tag="t1")
                nc.vector.tensor_tensor(t1, qnt, std[:, cc, :], OP.mult)
                # out = (qn*std + bs) + mean
                nc.vector.scalar_tensor_tensor(
                    out=t1, in0=t1, scalar=bs_sb[:, cc:cc + 1], in1=mean[cc],
                    op0=OP.add, op1=OP.add)
                nc.sync.dma_start(out_d[ts(cc, 128), ts(g, GN)], t1)

        for g in range(NG):
            pm = [psum_av.tile([128, GN], f32, tag="av", name=f"pm{g}_{i}")
                  for i in range(CC)]
            p2 = [psum_av.tile([128, GN], f32, tag="av", name=f"p2{g}_{i}")
                  for i in range(CC)]
            pcs = psum_cs.tile([1, GN], f32, tag="cs", name=f"pcs{g}")
            at_t = {}
            accs = {}

            def preadd(j, at):
                # column-sum pre-adds: window w owns chunks 8w..8w+7,
                # split into two 4-chunk branches that run on DVE and
                # GpSimd concurrently (3 chained adds each), then one
                # combine. Keeps the chain latency after the window's
                # last chunk to ~2 adds so cs_mm never stalls the PE.
                w, ph = divmod(j, CSW)
                half, hp = divmod(ph, 4)
                engA = nc.gpsimd if w % 2 == 0 else nc.vector
                engB = nc.vector if w % 2 == 0 else nc.gpsimd
                eng = engA if half == 0 else engB
                if hp == 0:
                    at_first[w, half] = at
                elif hp == 1:
                    acc = csacc.tile([128, GN], f32r,
                                     tag="accA" if half == 0 else "accB",
                                     bufs=2 if half == 0 else 1,
                                     name=f"acc{g}_{w}_{half}")
                    halfacc[w, half] = acc
                    eng.tensor_tensor(acc, at_first.pop((w, half)).bitcast(f32),
                                      at.bitcast(f32), OP.add)
                else:
                    eng.tensor_tensor(halfacc[w, half],
                                      halfacc[w, half].bitcast(f32),
                                      at.bitcast(f32), OP.add)
                if ph == CSW - 1:
                    a = halfacc.pop((w, 0))
                    b = halfacc.pop((w, 1))
                    engA.tensor_tensor(a, a.bitcast(f32), b.bitcast(f32),
                                       OP.add)
                    accs[w] = a

            def av_chunk(j):
                at = at_t.pop(j)
                first, last = (j == 0), (j == MC - 1)
                nc.tensor.matmul(pm[0], se_sb[:, j, 0:128], at,
                                 start=first, stop=last)
                nc.tensor.matmul(pm[1], se_sb[:, j, 128:256], at,
                                 start=first, stop=last)
                nc.tensor.matmul(p2[0], se2_sb[:, j, 0:128], at,
                                 start=first, stop=last)
                nc.tensor.matmul(p2[1], se2_sb[:, j, 128:256], at,
                                 start=first, stop=last)
                preadd(j, at)

            def cs_mm(w):
                nc.tensor.matmul(pcs, ones_col, accs.pop(w),
                                 start=(w == 0), stop=(w == NW - 1))

            at_first = {}
            halfacc = {}
            for mc in range(MC + AVD):
                if g == 0 and 4 <= mc < 7:
                    qe_tile(mc - 3)
                if mc == 10 and (g - 1) in pend:
                    epilogue_tail(g - 1)
                if mc < MC:
                    ps = psum_qk.tile([128, GN], f32, tag="qk",
                                      name=f"qk{g}_{mc}")
                    for cc in range(CC):
                        nc.tensor.matmul(ps, ke_sb[:, cc, ts(mc, 128)],
                                         qe_sb[:, cc, ts(g, GN)],
                                         start=(cc == 0), stop=(cc == CC - 1))
                    at = big.tile([128, GN], f32r, tag="big",
                                  name=f"at{g}_{mc}")
                    nc.scalar.activation(out=at, in_=ps, func=AF.Exp,
                                         bias=shift_t)
                    at_t[mc] = at
                if mc >= AVD:
                    av_chunk(mc - AVD)
                    jj = mc - AVD - CS_LAG
                    if jj >= 0 and jj % CSW == CSW - 1 and jj // CSW in accs:
                        cs_mm(jj // CSW)
            for w in sorted(accs):
                cs_mm(w)

            # ---- epilogue part 1 ----
            ivt = epi1.tile([128, GN], f32, tag="msq", name=f"ivr{g}")
            invrow = ivt[0:1, :]
            nc.vector.reciprocal_approx_fast(out=invrow, in_=pcs[0:1, :])
            mean = [epi.tile([128, GN], f32, tag="mean", name=f"u{g}_{cc}")
                    for cc in range(CC)]
            m2n = [epi1.tile([128, GN], f32, tag="m2n", name=f"v{g}_{cc}")
                   for cc in range(CC)]
            var2 = epi1.tile([128, CC, GN], f32, tag="var2", name=f"var{g}")
            std = epi1.tile([128, CC, GN], f32, tag="std", name=f"std{g}")
            if g < NG - 1:
                invb = invp.tile([128, GN], f32, tag="invb", name=f"ivb{g}")
                nc.gpsimd.partition_broadcast(invb, invrow)
                # normalize straight out of psum (no drain copies): the
                # AVD runway of the next group covers the bank release;
                # pm banks are freed first (AV(g+1) needs those soonest)
                for cc in range(CC):
                    nc.vector.tensor_tensor(mean[cc], pm[cc], invb, OP.mult)
                for cc in range(CC):
                    nc.vector.tensor_tensor(m2n[cc], p2[cc], invb, OP.mult)
                for cc in range(CC):
                    msq = epi1.tile([128, GN], f32, tag="msq")
                    nc.vector.tensor_tensor(msq, mean[cc], mean[cc], OP.mult)
                    nc.vector.tensor_tensor(var2[:, cc, :], m2n[cc], msq,
                                            OP.subtract)
                # floor > 0 so ln/exp of an exact-zero variance stays finite
                nc.vector.tensor_scalar_max(var2, var2, 1e-30)
                pend[g] = (var2, std, mean, invb)
            else:
                # last group: latency-optimized per-cc pipeline. 1/Z is
                # broadcast by the (now idle) PE as ones_row x invrow into
                # the freed pcs bank, then copied to SBUF by the scalar
                # engine — GpSimd stays off the critical tail.
                invb_ps = psum_cs.tile([128, GN], f32, tag="cs",
                                       name="invbps")
                nc.tensor.matmul(invb_ps, ones_row[0:1, :], invrow,
                                 start=True, stop=True)
                invb = invp.tile([128, GN], f32, tag="invb", name=f"ivb{g}")
                nc.scalar.copy(invb, invb_ps)
                qnt = []
                for cc in range(CC):
                    qn = epi1.tile([128, GN], f32, tag="qnt" if cc == 0 else "qnt1",
                                   name=f"qnt{g}_{cc}")
                    nc.vector.tensor_scalar(
                        out=qn, in0=ql_sb[:, cc, ts(g, GN)].bitcast(f32),
                        scalar1=mu_q[cc], scalar2=rs_q[cc],
                        op0=OP.subtract, op1=OP.mult)
                    qnt.append(qn)
                for cc in range(CC):
                    nc.vector.tensor_tensor(mean[cc], pm[cc], invb, OP.mult)
                    nc.vector.tensor_tensor(m2n[cc], p2[cc], invb, OP.mult)
                    msq = epi1.tile([128, GN], f32, tag="msq")
                    nc.vector.tensor_tensor(msq, mean[cc], mean[cc], OP.mult)
                    nc.vector.tensor_tensor(var2[:, cc, :], m2n[cc], msq,
                                            OP.subtract)
                    nc.vector.tensor_scalar_max(var2[:, cc, :],
                                                var2[:, cc, :], 1e-30)
                    nc.scalar.activation(out=var2[:, cc, :],
                                         in_=var2[:, cc, :], func=AF.Ln)
                    nc.scalar.activation(out=std[:, cc, :], in_=var2[:, cc, :],
                                         func=AF.Exp, scale=0.5)
                for cc in range(CC):
                    t1 = epi.tile([128, 512], f32, tag="t1")
                    nc.vector.tensor_tensor(t1, qnt[cc], std[:, cc, :],
                                            OP.mult)
                    nc.vector.scalar_tensor_tensor(
                        out=t1, in0=t1, scalar=bs_sb[:, cc:cc + 1],
                        in1=mean[cc], op0=OP.add, op1=OP.add)
                    # exposed tail: split across 4 DMA queues
                    for s in range(4):
                        nc.sync.dma_start(
                            out_d[ts(cc, 128), ds(g * GN + s * 128, 128)],
                            t1[:, ts(s, 128)])

    nc.finalize()
    return nc


_NC = None


def _get_nc():
    global _NC
    if _NC is None:
        _ensure_axon_hooks_stub()
        _NC = build_bass()
    return _NC


def make_in_maps(q, k, w_q, b_q, w_k, b_k, w_s, b_s):
    q = np.ascontiguousarray(np.asarray(q, dtype=np.float32))
    k = np.ascontiguousarray(np.asarray(k, dtype=np.float32))
    wqT = np.ascontiguousarray(np.asarray(w_q, np.float32).T)
    wkT = np.ascontiguousarray(np.asarray(w_k, np.float32).T)
    wsT = np.ascontiguousarray(np.asarray(w_s, np.float32).T)
    bq = np.ascontiguousarray(np.asarray(b_q, np.float32))
    bs = np.ascontiguousarray(np.asarray(b_s, np.float32))
    in_maps = []
    for core in range(8):
        b, h = divmod(core, 2)
        in_maps.append({
            "ql": np.ascontiguousarray(q[b][:, h * NLOC:(h + 1) * NLOC]),
            "qo": np.ascontiguousarray(q[b][:, (1 - h) * NLOC:(2 - h) * NLOC]),
            "kf": np.ascontiguousarray(k[b]),
            "wqT": wqT, "wkT": wkT, "wsT": wsT,
            "bq": bq, "bs": bs,
        })
    return in_maps


def kernel(**inputs):
    _ensure_axon_hooks_stub()
    from concourse.bass_utils import run_bass_kernel_spmd

    nc = _get_nc()
    in_maps = make_in_maps(**inputs)
    res = run_bass_kernel_spmd(nc, in_maps, core_ids=list(range(8)))
    out = np.empty((B, C, N), np.float32)
    for core in range(8):
        b, h = divmod(core, 2)
        out[b][:, h * NLOC:(h + 1) * NLOC] = res.results[core]["out"]
    return out


if __name__ == "__main__":
    import reference
    inputs = {k_: np.asarray(v) for k_, v in reference.setup_inputs().items()}
    expected = np.asarray(reference.reference(**inputs))
    actual = kernel(**inputs)
    err = np.linalg.norm(actual - expected) / np.linalg.norm(expected)
    print("Relative error:", err)
